# revision 1
# baseline (speedup 1.0000x reference)
"""LoRA QKV projection kernel for 8 Trainium2 NeuronCores.

Reference computation (per problem):
    qkv = x @ Wqkv^T + bqkv + concat(x@Aq^T@Bq^T, x@Ak^T@Bk^T, x@Av^T@Bv^T)

Strategy:
  * Host folds the rank-16 LoRA factors into the dense weight
    (W_eff = Wqkv + blockdiag(BqAq, BkAk, BvAv) — ~56 MFLOP, 0.05% of the
    116 GFLOP GEMM), so the device runs one pure GEMM at the roofline ridge.
  * Data-parallel: batch dim (8) sharded 1:1 over the 8 cores.
    Each core: y[4096, 2304] = x_b[4096, 768] @ W_eff^T + b.
  * Host pre-packs both operands K-major ([128, 6, fdim]) so every DMA is
    contiguous >=2KB runs and the device does zero transposes.
  * Matmuls run in float32r (fp32 storage, full PE rate at N>=256).
  * Raw-bass pipeline (explicit semaphores): this walrus build allows only
    ONE embedded sync-wait per compute instruction, which Tile's auto-sem
    placement violates; standalone sequencer wait_ge commands don't have
    that limit.
  * PSUM -> SBUF eviction fused with the bias add on the DVE; x loads on
    the sync (SP) HWDGE queue, y stores on the scalar (ACT) HWDGE queue so
    prefetch never queues behind stores.
"""

from contextlib import ExitStack

import numpy as np

import concourse.bass as bass
import concourse.mybir as mybir
from concourse.bass_utils import run_bass_kernel_spmd

P = 128
DIM = 768
NOUT = 3 * DIM          # 2304
KT = DIM // P           # 6 k-tiles
B = 8                   # batch == n_cores
M = 64 * 64             # 4096 tokens per core
TG = 512                # token supertile (x DMA granularity)
NGROUPS = M // TG       # 8
MT_PER_G = TG // P      # 4 m-tiles per supertile
N_CHUNKS = [(0, 512), (512, 512), (1024, 512), (1536, 512), (2048, 256)]
NCH = len(N_CHUNKS)     # 5 chunks per m-tile
N_PSUM = 6              # psum banks rotated across chunks
N_OBUF = 3              # output staging buffers

_F32 = mybir.dt.float32
_F32R = mybir.dt.float32r


def _build_program(reps=1):
    nc = bass.Bass()
    xt = nc.dram_tensor("xt", [P, KT, M], _F32R, kind="ExternalInput")
    wt = nc.dram_tensor("wt", [P, KT, NOUT], _F32R, kind="ExternalInput")
    bi = nc.dram_tensor("bias", [P, NOUT], _F32, kind="ExternalInput")
    y = nc.dram_tensor("y", [M, NOUT], _F32, kind="ExternalOutput")

    with ExitStack() as ctx:
        wt_sb = ctx.enter_context(nc.sbuf_tensor("wt_sb", [P, KT, NOUT], _F32R))
        bias_sb = ctx.enter_context(nc.sbuf_tensor("bias_sb", [P, NOUT], _F32))
        x_sb = [
            ctx.enter_context(nc.sbuf_tensor(f"x_sb{i}", [P, KT, TG], _F32R))
            for i in range(2)
        ]
        o_sb = [
            ctx.enter_context(nc.sbuf_tensor(f"o_sb{i}", [P, NOUT], _F32))
            for i in range(N_OBUF)
        ]
        ps = [
            ctx.enter_context(nc.psum_tensor(f"ps{i}", [P, 512], _F32))
            for i in range(N_PSUM)
        ]
        s_x = ctx.enter_context(nc.semaphore("s_x"))
        s_w = ctx.enter_context(nc.semaphore("s_w"))
        s_b = ctx.enter_context(nc.semaphore("s_b"))
        s_mm = ctx.enter_context(nc.semaphore("s_mm"))
        s_tt = ctx.enter_context(nc.semaphore("s_tt"))
        s_out = ctx.enter_context(nc.semaphore("s_out"))
        block = ctx.enter_context(nc.Block())

        @block.sync
        def _(sync):
            for k in range(KT):
                if k >= 1:
                    sync.wait_ge(s_w, 16 * k)
                sync.dma_start(
                    out=wt_sb[:, k : k + 1, :], in_=wt[:, k : k + 1, :]
                ).then_inc(s_w, 16)
            for ga in range(NGROUPS * reps):
                g = ga % NGROUPS
                if ga >= 1:
                    # self-throttle: previous x DMA retired (sem-race rule)
                    sync.wait_ge(s_x, 16 * ga)
                if ga >= 2:
                    # x ping-pong slot free once supertile ga-2 fully evicted
                    sync.wait_ge(s_tt, NCH * MT_PER_G * (ga - 1))
                sync.dma_start(
                    out=x_sb[ga % 2][:], in_=xt[:, :, g * TG : (g + 1) * TG]
                ).then_inc(s_x, 16)

        @block.scalar
        def _(scalar):
            scalar.dma_start(out=bias_sb[:], in_=bi[:]).then_inc(s_b, 16)
            for ma in range(NGROUPS * MT_PER_G * reps):
                m = ma % (NGROUPS * MT_PER_G)
                if ma >= 1:
                    scalar.wait_ge(s_out, 16 * ma)
                scalar.wait_ge(s_tt, NCH * (ma + 1))
                scalar.dma_start(
                    out=y[m * P : (m + 1) * P, :], in_=o_sb[ma % N_OBUF][:]
                ).then_inc(s_out, 16)

        @block.tensor
        def _(tensor):
            c = 0
            for ga in range(NGROUPS * reps):
                tensor.wait_ge(s_x, 16 * (ga + 1))
                for ms in range(MT_PER_G):
                    for n0, nsz in N_CHUNKS:
                        if c >= N_PSUM:
                            # DVE finished reading this psum bank
                            tensor.wait_ge(s_tt, c - N_PSUM + 1)
                        for k in range(KT):
                            if c == 0:
                                tensor.wait_ge(s_w, 16 * (k + 1))
                            mm = nc.tensor.matmul(
                                ps[c % N_PSUM][:, :nsz],
                                lhsT=x_sb[ga % 2][:, k, ms * P : (ms + 1) * P],
                                rhs=wt_sb[:, k, n0 : n0 + nsz],
                                start=(k == 0),
                                stop=(k == KT - 1),
                            )
                        mm.then_inc(s_mm, 1)
                        c += 1

        @block.vector
        def _(vector):
            vector.wait_ge(s_b, 16)
            c = 0
            for ma in range(NGROUPS * MT_PER_G * reps):
                for j, (n0, nsz) in enumerate(N_CHUNKS):
                    vector.wait_ge(s_mm, c + 1)
                    if j == 0 and ma >= N_OBUF:
                        # o_sb slot free once the ma-3 store retired
                        vector.wait_ge(s_out, 16 * (ma - N_OBUF + 1))
                    nc.vector.tensor_add(
                        o_sb[ma % N_OBUF][:, n0 : n0 + nsz],
                        ps[c % N_PSUM][:, :nsz],
                        bias_sb[:, n0 : n0 + nsz],
                    ).then_inc(s_tt, 1)
                    c += 1

    return nc


def _prepare_inputs(x, Wqkv, bqkv, Aq, Bq, Ak, Bk, Av, Bv):
    x = np.asarray(x, dtype=np.float32)
    Wqkv = np.asarray(Wqkv, dtype=np.float32)
    bqkv = np.asarray(bqkv, dtype=np.float32)

    # Fold LoRA: W_eff[j-th slice] = Wqkv[j-th slice] + B_j @ A_j
    w_eff = Wqkv.copy()
    for j, (A, Bm) in enumerate(((Aq, Bq), (Ak, Bk), (Av, Bv))):
        A = np.asarray(A, dtype=np.float32)
        Bm = np.asarray(Bm, dtype=np.float32)
        w_eff[j * DIM : (j + 1) * DIM] += Bm @ A

    # K-major packing: [p, k, f] = T[f, k*128 + p] for T in {x_b, W_eff}.
    wt = np.ascontiguousarray(w_eff.reshape(NOUT, KT, P).transpose(2, 1, 0))
    bias_rep = np.ascontiguousarray(np.broadcast_to(bqkv, (P, NOUT)))

    in_maps = []
    for b in range(B):
        xb = x[b].reshape(M, KT, P)
        xtb = np.ascontiguousarray(xb.transpose(2, 1, 0))  # [128, 6, 4096]
        in_maps.append({"xt": xtb, "wt": wt, "bias": bias_rep})
    return in_maps


def _run(inputs, trace=False, trace_kwargs=None):
    nc = _build_program()
    in_maps = _prepare_inputs(**inputs)
    res = run_bass_kernel_spmd(
        nc,
        in_maps,
        core_ids=list(range(B)),
        trace=trace,
        **(trace_kwargs or {}),
    )
    outs = res.results
    y = np.stack([np.asarray(outs[b]["y"]).reshape(64, 64, NOUT) for b in range(B)])
    return y, res


def kernel(**inputs):
    y, _ = _run(inputs, trace=False)
    return y



# revision 7
# speedup vs baseline: 1.1117x; 1.1117x over previous
"""LoRA QKV projection kernel for 8 Trainium2 NeuronCores.

Reference computation (per problem):
    qkv = x @ Wqkv^T + bqkv + concat(x@Aq^T@Bq^T, x@Ak^T@Bk^T, x@Av^T@Bv^T)

Strategy:
  * Host folds the rank-16 LoRA factors into the dense weight
    (W_eff = Wqkv + blockdiag(BqAq, BkAk, BvAv) — ~56 MFLOP, 0.05% of the
    116 GFLOP GEMM), so the device runs one pure GEMM at the roofline ridge.
  * Data-parallel: batch dim (8) sharded 1:1 over the 8 cores.
    Each core: y[4096, 2304] = x_b[4096, 768] @ W_eff^T.
  * bf16 everywhere off-chip: x and W_eff are cast to bf16 on host (halves
    input DMA, enables fast-weight-load on the PE), the GEMM accumulates in
    fp32 PSUM, and the output is stored as bf16 (halves store DMA). Host
    upcasts to fp32 and adds the bias during the unshard (exact in fp32).
    Measured rel-l2 vs fp32 reference: ~2.6e-3, far under the 2e-2 gate.
  * Whole x (6.3 MB bf16) and W (3.5 MB bf16) are SBUF-resident; x streams
    on the sync (SP) HWDGE queue while W pieces + y stores share the
    scalar (ACT) queue, so weight prefetch never queues behind stores.
  * DMA completion semaphores: a DMA's +16 arrives as 16 independent
    per-SDMA-engine +1s, so cumulative waits on a shared semaphore are
    UNSOUND with >1 DMA of that semaphore in flight (engines race ahead).
    Each DMA stream uses K=3 rotating semaphores with a producer-side
    throttle (issue of DMA j waits for DMA j-K's completion), so each
    semaphore has at most one in-flight incrementer and full-count waits
    are exact. Store completions rotate 2 sems by supertile parity (the
    next same-parity store group is data-dependent on the wait, so no
    throttle needed).
  * Supertile loop is chunk-column-major so the first m-tiles only need
    the first 512 W columns; first x supertile and the first W column
    chunk are split fine so the PE starts ~3 us in.
  * Raw-bass pipeline (explicit semaphores); PSUM->SBUF eviction (with the
    fp32->bf16 cast) on the DVE rotates through all 8 PSUM banks.
  * The first start-of-group N=512 matmul is emitted twice: this walrus
    build eats one early MATMUL (verified via the NTFF instruction
    stream); the duplicate is idempotent (start=True clears the bank).
"""

from contextlib import ExitStack

import ml_dtypes
import numpy as np

import concourse.bass as bass
import concourse.mybir as mybir
from concourse.bass_utils import run_bass_kernel_spmd

P = 128
DIM = 768
NOUT = 3 * DIM          # 2304
KT = DIM // P           # 6 k-tiles
B = 8                   # batch == n_cores
M = 64 * 64             # 4096 tokens per core
MT = M // P             # 32 m-tiles per core
TG = 512                # token supertile
NG = M // TG            # 8 supertiles
MT_G = TG // P          # 4 m-tiles per supertile
N_CHUNKS = [(0, 512), (512, 512), (1024, 512), (1536, 512), (2048, 256)]
NCH = len(N_CHUNKS)     # 5 chunk columns
N_PSUM = 8              # all psum banks
OB = 2                  # output staging buffers (supertile granularity)
KSEM = 3                # rotating DMA-completion sems per input stream

_F32 = mybir.dt.float32
_BF16 = mybir.dt.bfloat16


def _build_program():
    nc = bass.Bass()
    xt = nc.dram_tensor("xt", [P, MT, KT, P], _BF16, kind="ExternalInput")
    wt = nc.dram_tensor("wt", [P, KT, NOUT], _BF16, kind="ExternalInput")
    y = nc.dram_tensor("y", [M, NOUT], _BF16, kind="ExternalOutput")

    with ExitStack() as ctx:
        x_sb = ctx.enter_context(nc.sbuf_tensor("x_sb", [P, MT, KT, P], _BF16))
        wt_sb = ctx.enter_context(nc.sbuf_tensor("wt_sb", [P, KT, NOUT], _BF16))
        o_sb = ctx.enter_context(nc.sbuf_tensor("o_sb", [P, OB, MT_G, NOUT], _BF16))
        ps = [
            ctx.enter_context(nc.psum_tensor(f"ps{i}", [P, 512], _F32))
            for i in range(N_PSUM)
        ]
        sx = [ctx.enter_context(nc.semaphore(f"sx{i}")) for i in range(KSEM)]
        sw = [ctx.enter_context(nc.semaphore(f"sw{i}")) for i in range(KSEM)]
        so = [ctx.enter_context(nc.semaphore(f"so{i}")) for i in range(OB)]
        s_mm = ctx.enter_context(nc.semaphore("s_mm"))
        s_tt = ctx.enter_context(nc.semaphore("s_tt"))
        block = ctx.enter_context(nc.Block())

        # x milestones: j=0..3 granules of supertile 0, j=4..10 supertiles 1..7
        # w milestones: j=0..5 k-granules of chunk-col 0, j=6..9 pieces c=1..4
        def x_done(j):
            return sx[j % KSEM], 16 * (j // KSEM + 1)

        def w_done(j):
            return sw[j % KSEM], 16 * (j // KSEM + 1)

        @block.sync
        def _(sync):
            for j in range(MT_G + NG - 1):
                if j >= KSEM:
                    sem, val = x_done(j - KSEM)
                    sync.wait_ge(sem, val)
                if j < MT_G:
                    d = sync.dma_start(out=x_sb[:, j], in_=xt[:, j])
                else:
                    g = j - MT_G + 1
                    d = sync.dma_start(
                        out=x_sb[:, MT_G * g : MT_G * (g + 1)],
                        in_=xt[:, MT_G * g : MT_G * (g + 1)],
                    )
                d.then_inc(sx[j % KSEM], 16)

        @block.scalar
        def _(scalar):
            n0c0, nszc0 = N_CHUNKS[0]
            for j in range(KT + NCH - 1):
                if j >= KSEM:
                    sem, val = w_done(j - KSEM)
                    scalar.wait_ge(sem, val)
                if j < KT:
                    d = scalar.dma_start(
                        out=wt_sb[:, j, n0c0 : n0c0 + nszc0],
                        in_=wt[:, j, n0c0 : n0c0 + nszc0],
                    )
                else:
                    n0, nsz = N_CHUNKS[j - KT + 1]
                    d = scalar.dma_start(
                        out=wt_sb[:, :, n0 : n0 + nsz], in_=wt[:, :, n0 : n0 + nsz]
                    )
                d.then_inc(sw[j % KSEM], 16)
            # y stores (behind the W pieces on the same FIFO queue).
            for g in range(NG):
                for ms in range(MT_G):
                    ma = MT_G * g + ms
                    # all NCH chunk-columns of this m-tile evicted
                    scalar.wait_ge(s_tt, NCH * MT_G * g + (NCH - 1) * MT_G + ms + 1)
                    scalar.dma_start(
                        out=y[ma * P : (ma + 1) * P, :], in_=o_sb[:, g % OB, ms, :]
                    ).then_inc(so[g % OB], 16)

        @block.tensor
        def _(tensor):
            cyc = 0
            for g in range(NG):
                for c, (n0, nsz) in enumerate(N_CHUNKS):
                    for ms in range(MT_G):
                        ma = MT_G * g + ms
                        if g == 0:
                            if c == 0:
                                sem, val = x_done(ms)
                                tensor.wait_ge(sem, val)
                            elif ms == 0:
                                sem, val = w_done(KT + c - 1)
                                tensor.wait_ge(sem, val)
                        elif c == 0 and ms == 0:
                            sem, val = x_done(MT_G + g - 1)
                            tensor.wait_ge(sem, val)
                        if cyc >= N_PSUM:
                            # DVE finished reading this psum bank
                            tensor.wait_ge(s_tt, cyc - N_PSUM + 1)
                        for k in range(KT):
                            if g == 0 and c == 0 and ms == 0:
                                sem, val = w_done(k)
                                tensor.wait_ge(sem, val)
                                if k == 0:
                                    # Sacrificial duplicate of the k=0
                                    # matmul: this walrus build eats the
                                    # first start-of-group N=512 MATMUL
                                    # (observed in the NTFF stream across
                                    # three variants). start=True clears
                                    # the bank, so whichever copy survives
                                    # the result is correct.
                                    nc.tensor.matmul(
                                        ps[0][:, :nsz],
                                        lhsT=x_sb[:, 0, 0, :],
                                        rhs=wt_sb[:, 0, n0 : n0 + nsz],
                                        start=True,
                                        stop=False,
                                        skip_group_check=True,
                                    )
                            mm = nc.tensor.matmul(
                                ps[cyc % N_PSUM][:, :nsz],
                                lhsT=x_sb[:, ma, k, :],
                                rhs=wt_sb[:, k, n0 : n0 + nsz],
                                start=(k == 0),
                                stop=(k == KT - 1),
                            )
                        mm.then_inc(s_mm, 1)
                        cyc += 1

        @block.vector
        def _(vector):
            cyc = 0
            for g in range(NG):
                for c, (n0, nsz) in enumerate(N_CHUNKS):
                    for ms in range(MT_G):
                        vector.wait_ge(s_mm, cyc + 1)
                        if c == 0 and ms == 0 and g >= OB:
                            # staging buffer free once supertile g-OB stored
                            vector.wait_ge(so[g % OB], 16 * MT_G * (g // OB))
                        nc.vector.tensor_copy(
                            o_sb[:, g % OB, ms, n0 : n0 + nsz],
                            ps[cyc % N_PSUM][:, :nsz],
                        ).then_inc(s_tt, 1)
                        cyc += 1

    return nc


def _prepare_inputs(x, Wqkv, bqkv, Aq, Bq, Ak, Bk, Av, Bv):
    x = np.asarray(x, dtype=np.float32)
    Wqkv = np.asarray(Wqkv, dtype=np.float32)

    # Fold LoRA: W_eff[j-th slice] = Wqkv[j-th slice] + B_j @ A_j
    w_eff = Wqkv.copy()
    for j, (A, Bm) in enumerate(((Aq, Bq), (Ak, Bk), (Av, Bv))):
        A = np.asarray(A, dtype=np.float32)
        Bm = np.asarray(Bm, dtype=np.float32)
        w_eff[j * DIM : (j + 1) * DIM] += Bm @ A

    # wt[p, k, n] = W_eff[n, k*128+p], bf16
    wt = np.ascontiguousarray(
        w_eff.reshape(NOUT, KT, P).transpose(2, 1, 0).astype(ml_dtypes.bfloat16)
    )

    in_maps = []
    for b in range(B):
        # xt[p, ma, k, t] = x[b, ma*128+t, k*128+p], bf16
        xb = x[b].reshape(MT, P, KT, P)
        xtb = np.ascontiguousarray(
            xb.transpose(3, 0, 2, 1).astype(ml_dtypes.bfloat16)
        )
        in_maps.append({"xt": xtb, "wt": wt})
    return in_maps


def _run(inputs, trace=False, trace_kwargs=None):
    nc = _build_program()
    in_maps = _prepare_inputs(**inputs)
    res = run_bass_kernel_spmd(
        nc,
        in_maps,
        core_ids=list(range(B)),
        trace=trace,
        **(trace_kwargs or {}),
    )
    bqkv = np.asarray(inputs["bqkv"], dtype=np.float32)
    outs = res.results
    y = np.stack(
        [
            np.asarray(outs[b]["y"]).astype(np.float32).reshape(64, 64, NOUT) + bqkv
            for b in range(B)
        ]
    )
    return y, res


def kernel(**inputs):
    y, _ = _run(inputs, trace=False)
    return y


# revision 9
# speedup vs baseline: 1.1836x; 1.0647x over previous
"""LoRA QKV projection kernel for 8 Trainium2 NeuronCores.

Reference computation (per problem):
    qkv = x @ Wqkv^T + bqkv + concat(x@Aq^T@Bq^T, x@Ak^T@Bk^T, x@Av^T@Bv^T)

Strategy:
  * Host folds the rank-16 LoRA factors into the dense weight
    (W_eff = Wqkv + blockdiag(BqAq, BkAk, BvAv) — ~56 MFLOP, 0.05% of the
    116 GFLOP GEMM), so the device runs one pure GEMM at the roofline ridge.
  * Data-parallel: batch dim (8) sharded 1:1 over the 8 cores.
    Each core: y[4096, 2304] = x_b[4096, 768] @ W_eff^T.
  * bf16 everywhere off-chip: x and W_eff are cast to bf16 on host (halves
    input DMA, enables fast-weight-load on the PE), the GEMM accumulates in
    fp32 PSUM, and the output is stored as bf16 (halves store DMA). Host
    upcasts to fp32 and adds the bias during the unshard (exact in fp32).
    Measured rel-l2 vs fp32 reference: ~2.6e-3, far under the 2e-2 gate.
  * Whole x (6.3 MB bf16) and W (3.5 MB bf16) are SBUF-resident; x streams
    on the sync (SP) HWDGE queue while W pieces + y stores share the
    scalar (ACT) queue, so weight prefetch never queues behind stores.
  * DMA completion semaphores: a DMA's +16 arrives as 16 independent
    per-SDMA-engine +1s, so cumulative waits on a shared semaphore are
    UNSOUND with >1 DMA of that semaphore in flight (engines race ahead).
    Each DMA stream uses K=3 rotating semaphores with a producer-side
    throttle (issue of DMA j waits for DMA j-K's completion), so each
    semaphore has at most one in-flight incrementer and full-count waits
    are exact. Store completions rotate 2 sems by supertile parity (the
    next same-parity store group is data-dependent on the wait, so no
    throttle needed).
  * Supertile loop is chunk-column-major so the first m-tiles only need
    the first 512 W columns; first x supertile and the first W column
    chunk are split fine so the PE starts ~3 us in.
  * Raw-bass pipeline (explicit semaphores); PSUM->SBUF eviction (with the
    fp32->bf16 cast) on the DVE rotates through all 8 PSUM banks.
  * The first start-of-group N=512 matmul is emitted twice: this walrus
    build eats one early MATMUL (verified via the NTFF instruction
    stream); the duplicate is idempotent (start=True clears the bank).
"""

from contextlib import ExitStack

import ml_dtypes
import numpy as np

import concourse.bass as bass
import concourse.mybir as mybir
from concourse.bass_utils import run_bass_kernel_spmd

P = 128
DIM = 768
NOUT = 3 * DIM          # 2304
KT = DIM // P           # 6 k-tiles
B = 8                   # batch == n_cores
M = 64 * 64             # 4096 tokens per core
MT = M // P             # 32 m-tiles per core
TG = 512                # token supertile
NG = M // TG            # 8 supertiles
MT_G = TG // P          # 4 m-tiles per supertile
N_CHUNKS = [(0, 512), (512, 512), (1024, 512), (1536, 512), (2048, 256)]
NCH = len(N_CHUNKS)     # 5 chunk columns
N_PSUM = 8              # all psum banks
OB = 3                  # output staging buffers (supertile granularity)
KSEM = 3                # rotating DMA-completion sems per input stream

_F32 = mybir.dt.float32
_BF16 = mybir.dt.bfloat16


def _build_program():
    nc = bass.Bass()
    xt = nc.dram_tensor("xt", [P, MT, KT, P], _BF16, kind="ExternalInput")
    wt = nc.dram_tensor("wt", [P, KT, NOUT], _BF16, kind="ExternalInput")
    y = nc.dram_tensor("y", [M, NOUT], _BF16, kind="ExternalOutput")

    with ExitStack() as ctx:
        x_sb = ctx.enter_context(nc.sbuf_tensor("x_sb", [P, MT, KT, P], _BF16))
        wt_sb = ctx.enter_context(nc.sbuf_tensor("wt_sb", [P, KT, NOUT], _BF16))
        o_sb = ctx.enter_context(nc.sbuf_tensor("o_sb", [P, OB, MT_G, NOUT], _BF16))
        ps = [
            ctx.enter_context(nc.psum_tensor(f"ps{i}", [P, 512], _F32))
            for i in range(N_PSUM)
        ]
        sx = [ctx.enter_context(nc.semaphore(f"sx{i}")) for i in range(KSEM)]
        sw = [ctx.enter_context(nc.semaphore(f"sw{i}")) for i in range(KSEM)]
        so = [ctx.enter_context(nc.semaphore(f"so{i}")) for i in range(OB)]
        s_mm = ctx.enter_context(nc.semaphore("s_mm"))
        s_tt = ctx.enter_context(nc.semaphore("s_tt"))
        block = ctx.enter_context(nc.Block())

        # x milestones: j=0..3 granules of supertile 0, j=4..10 supertiles 1..7
        # w milestones: j=0..5 k-granules of chunk-col 0, j=6..9 pieces c=1..4
        def x_done(j):
            return sx[j % KSEM], 16 * (j // KSEM + 1)

        def w_done(j):
            return sw[j % KSEM], 16 * (j // KSEM + 1)

        @block.sync
        def _(sync):
            for j in range(MT_G + NG - 1):
                if j >= KSEM:
                    sem, val = x_done(j - KSEM)
                    sync.wait_ge(sem, val)
                if j < MT_G:
                    d = sync.dma_start(out=x_sb[:, j], in_=xt[:, j])
                else:
                    g = j - MT_G + 1
                    if g >= 2:
                        # Just-in-time: don't fight the W pieces for HBM
                        # bandwidth during the ramp. Supertile g is needed
                        # one full supertile (~26 us) after this fires.
                        sync.wait_ge(s_tt, NCH * MT_G * (g - 2) + 1)
                    d = sync.dma_start(
                        out=x_sb[:, MT_G * g : MT_G * (g + 1)],
                        in_=xt[:, MT_G * g : MT_G * (g + 1)],
                    )
                d.then_inc(sx[j % KSEM], 16)

        @block.scalar
        def _(scalar):
            n0c0, nszc0 = N_CHUNKS[0]
            for j in range(KT + NCH - 1):
                if j >= KSEM:
                    sem, val = w_done(j - KSEM)
                    scalar.wait_ge(sem, val)
                if j < KT:
                    d = scalar.dma_start(
                        out=wt_sb[:, j, n0c0 : n0c0 + nszc0],
                        in_=wt[:, j, n0c0 : n0c0 + nszc0],
                    )
                else:
                    n0, nsz = N_CHUNKS[j - KT + 1]
                    d = scalar.dma_start(
                        out=wt_sb[:, :, n0 : n0 + nsz], in_=wt[:, :, n0 : n0 + nsz]
                    )
                d.then_inc(sw[j % KSEM], 16)
            # y stores (behind the W pieces on the same FIFO queue).
            for g in range(NG):
                for ms in range(MT_G):
                    ma = MT_G * g + ms
                    # all NCH chunk-columns of this m-tile evicted
                    scalar.wait_ge(s_tt, NCH * MT_G * g + (NCH - 1) * MT_G + ms + 1)
                    scalar.dma_start(
                        out=y[ma * P : (ma + 1) * P, :], in_=o_sb[:, g % OB, ms, :]
                    ).then_inc(so[g % OB], 16)

        @block.tensor
        def _(tensor):
            cyc = 0
            for g in range(NG):
                for c, (n0, nsz) in enumerate(N_CHUNKS):
                    for ms in range(MT_G):
                        ma = MT_G * g + ms
                        if g == 0:
                            if c == 0:
                                sem, val = x_done(ms)
                                tensor.wait_ge(sem, val)
                            elif ms == 0:
                                sem, val = w_done(KT + c - 1)
                                tensor.wait_ge(sem, val)
                        elif c == 0 and ms == 0:
                            sem, val = x_done(MT_G + g - 1)
                            tensor.wait_ge(sem, val)
                        if cyc >= N_PSUM:
                            # DVE finished reading this psum bank
                            tensor.wait_ge(s_tt, cyc - N_PSUM + 1)
                        for k in range(KT):
                            if g == 0 and c == 0 and ms == 0:
                                sem, val = w_done(k)
                                tensor.wait_ge(sem, val)
                                if k == 0:
                                    # Sacrificial duplicate of the k=0
                                    # matmul: this walrus build eats the
                                    # first start-of-group N=512 MATMUL
                                    # (observed in the NTFF stream across
                                    # three variants). start=True clears
                                    # the bank, so whichever copy survives
                                    # the result is correct.
                                    nc.tensor.matmul(
                                        ps[0][:, :nsz],
                                        lhsT=x_sb[:, 0, 0, :],
                                        rhs=wt_sb[:, 0, n0 : n0 + nsz],
                                        start=True,
                                        stop=False,
                                        skip_group_check=True,
                                    )
                            mm = nc.tensor.matmul(
                                ps[cyc % N_PSUM][:, :nsz],
                                lhsT=x_sb[:, ma, k, :],
                                rhs=wt_sb[:, k, n0 : n0 + nsz],
                                start=(k == 0),
                                stop=(k == KT - 1),
                            )
                        mm.then_inc(s_mm, 1)
                        cyc += 1

        @block.vector
        def _(vector):
            cyc = 0
            for g in range(NG):
                for c, (n0, nsz) in enumerate(N_CHUNKS):
                    for ms in range(MT_G):
                        vector.wait_ge(s_mm, cyc + 1)
                        if c == 0 and ms == 0 and g >= OB:
                            # staging buffer free once supertile g-OB stored
                            vector.wait_ge(so[g % OB], 16 * MT_G * (g // OB))
                        nc.vector.tensor_copy(
                            o_sb[:, g % OB, ms, n0 : n0 + nsz],
                            ps[cyc % N_PSUM][:, :nsz],
                        ).then_inc(s_tt, 1)
                        cyc += 1

    return nc


def _prepare_inputs(x, Wqkv, bqkv, Aq, Bq, Ak, Bk, Av, Bv):
    x = np.asarray(x, dtype=np.float32)
    Wqkv = np.asarray(Wqkv, dtype=np.float32)

    # Fold LoRA: W_eff[j-th slice] = Wqkv[j-th slice] + B_j @ A_j
    w_eff = Wqkv.copy()
    for j, (A, Bm) in enumerate(((Aq, Bq), (Ak, Bk), (Av, Bv))):
        A = np.asarray(A, dtype=np.float32)
        Bm = np.asarray(Bm, dtype=np.float32)
        w_eff[j * DIM : (j + 1) * DIM] += Bm @ A

    # wt[p, k, n] = W_eff[n, k*128+p], bf16
    wt = np.ascontiguousarray(
        w_eff.reshape(NOUT, KT, P).transpose(2, 1, 0).astype(ml_dtypes.bfloat16)
    )

    in_maps = []
    for b in range(B):
        # xt[p, ma, k, t] = x[b, ma*128+t, k*128+p], bf16
        xb = x[b].reshape(MT, P, KT, P)
        xtb = np.ascontiguousarray(
            xb.transpose(3, 0, 2, 1).astype(ml_dtypes.bfloat16)
        )
        in_maps.append({"xt": xtb, "wt": wt})
    return in_maps


def _run(inputs, trace=False, trace_kwargs=None):
    nc = _build_program()
    in_maps = _prepare_inputs(**inputs)
    res = run_bass_kernel_spmd(
        nc,
        in_maps,
        core_ids=list(range(B)),
        trace=trace,
        **(trace_kwargs or {}),
    )
    bqkv = np.asarray(inputs["bqkv"], dtype=np.float32)
    outs = res.results
    y = np.stack(
        [
            np.asarray(outs[b]["y"]).astype(np.float32).reshape(64, 64, NOUT) + bqkv
            for b in range(B)
        ]
    )
    return y, res


def kernel(**inputs):
    y, _ = _run(inputs, trace=False)
    return y
